# revision 44
# baseline (speedup 1.0000x reference)
# MoE layer (all-experts dense MLP + weighted combine) on 8 TRN2 NeuronCores.
#
# Reference, for every token b (B=65536 total):
#   h_e   = relu(x @ W1[e] + b1[e])          e = 0..7
#   y_e   = h_e @ W2[e] + b2[e]
#   out_b = sum_e weights[b, e] * y_e
#
# Strategy (data-parallel over B, expert params replicated):
#   - Shard B across the 8 cores (8192 tokens each).
#   - Hidden dim stays on partitions ("hdim-major"):
#       L1:  z_e^T[h, b]  = W1_e^T @ x^T          (W1 chunks stationary)
#       h_e^T             = relu(z_e^T + b1_e)    (split ACT/DVE)
#       hs_e^T            = h_e^T * w_bcast_e     (DVE, per chunk)
#       out^T[o, b]       = sum_{e,k} W2_chunk^T @ hs_chunk + b2^T @ w^T
#     accumulated in two per-half PSUM banks - the expert combine is free
#     and consecutive tiles' accumulations overlap.
#   - w broadcast to 128 partitions via per-expert DMAs with a step-0
#     partition AP reading the host-transposed weights row from DRAM.
#     The relu ops deliberately depend only on z + b1 (no DMA), so the
#     z-PSUM drain never waits on the broadcast stream.
#   - Head/tail tuned: consts ride the scalar/gpsimd queues (w1 in pieces)
#     so tile-0's xt is the sync queue's first transfer; dummy matmuls in
#     the DMA-bound head release the HAM clock gate before real work; the
#     b2 seed matmuls are emitted after expert 0's z work.
#   - Host-side prep: x / weights transposed + cast to bf16, expert params
#     in matmul-ready layout; output produced transposed and un-transposed
#     on the host. All device work is the unavoidable compute.
import numpy as np
import ml_dtypes

import concourse.bass as bass
import concourse.mybir as mybir
import concourse.tile as tile
import concourse.bass_utils as _bu
from concourse.bass_utils import run_bass_kernel_spmd


E, D_IN, D_HID, D_OUT, B = 8, 128, 256, 128, 65536
N_CORES = 8
B_SHARD = B // N_CORES  # 8192
NB = 1024               # tokens per tile
NCHUNK = D_HID // 128   # 2 hidden-dim chunks per expert

BF16 = mybir.dt.bfloat16
F32 = mybir.dt.float32
RELU = mybir.ActivationFunctionType.Relu
MAX = mybir.AluOpType.max
MULT = mybir.AluOpType.mult

# Engine split knobs (tuned against the profile so every engine sits
# below the PE's ~14.3us/tile floor):
#  - FUSED_CHUNKS (c = 2*e + m): relu+weighting in one DVE
#    scalar_tensor_tensor, with the w*b1 term host-folded into b2'.
#    NOTE: measured counterproductive — the fused op puts the wbc DMA on
#    the critical z-drain path and the pipeline de-syncs (+33us). Keep ().
#  - DVE_RELU_CHUNKS: relu on DVE via tensor_scalar (z + b1 deps only).
#  - remaining chunks: ACT relu; per-expert weight multiply on DVE.
#  - GP_MULT_EXPERTS: multiplies offloaded to GPSIMD: measured
#    counterproductive (blocks wbc triggers in the gpsimd FIFO). Keep ().
FUSED_CHUNKS = ()
DVE_RELU_CHUNKS = (1, 5, 9)
GP_MULT_EXPERTS = ()
N_WARM_MM = 40  # dummy matmuls in the DMA-bound head to spin up HAM

_nc_cache = {}


def dedup_ldw(nc):
    """Drop redundant PE weight loads.

    Tile emits an InstLdweights before every InstMatmult; consecutive
    matmuls over the two 512-token halves of a tile reuse the same
    stationary weights, so the second load is a hardware no-op (weights
    persist in the PE array until the next load). Deleting it saves PE
    queue time; its semaphore waits/updates are carried onto the next PE
    instruction (legalize_waits splits any overflow afterwards).
    """
    for f in nc.m.functions:
        for b in f.blocks:
            il = b.instructions
            out = []
            last_key = None
            carry_w, carry_u = [], []
            for inst in il:
                if inst.engine != mybir.EngineType.PE:
                    out.append(inst)
                    continue
                if isinstance(inst, mybir.InstLdweights):
                    key = str(inst.ins[0])
                    if key == last_key:
                        si = inst.sync_info
                        if si is not None:
                            carry_w.extend(list(si.on_wait))
                            carry_u.extend(list(si.on_update))
                        continue
                    last_key = key
                elif not isinstance(
                    inst, (mybir.InstMatmult, mybir.InstEventSemaphore)
                ):
                    last_key = None
                if carry_w or carry_u:
                    si = inst.sync_info
                    w = (list(si.on_wait) if si else []) + carry_w
                    u = (list(si.on_update) if si else []) + carry_u
                    inst.sync_info = mybir.SyncInfo(on_wait=w, on_update=u)
                    carry_w, carry_u = [], []
                out.append(inst)
            il[:] = out
    return nc


def legalize_waits(nc):
    """Split multi-wait instructions into standalone EventSemaphore waits.

    The walrus build in this container enforces the hardware sync-slot
    budget strictly: a normal instruction holds at most 1 sem wait (+1
    update); an EventSemaphore instruction holds 2. Tile's scheduler
    attaches up to 3 waits per instruction (and ~11 on the kernel-tail
    drain), which codegen rejects with "Too many sync wait commands".
    Hoisting the excess waits into standalone EventSemaphore instructions
    immediately before the op (same engine queue, so they gate execution
    identically) makes the program legal without changing semantics.
    """
    for f in nc.m.functions:
        for b in f.blocks:
            il = b.instructions
            out = []
            changed = False
            for inst in il:
                si = inst.sync_info
                if si is not None:
                    waits = list(si.on_wait)
                    upds = list(si.on_update)
                    assert len(upds) <= 1, f"{inst.name}: {len(upds)} updates"
                    cap = 2 if isinstance(inst, mybir.InstEventSemaphore) else 1
                    if len(waits) > cap:
                        extra, keep = waits[:-cap], waits[-cap:]
                        k = 0
                        while extra:
                            chunk, extra = extra[:2], extra[2:]
                            ev = mybir.InstEventSemaphore(
                                name=f"{inst.name}-lw{k}", ins=[], outs=[]
                            )
                            ev.engine = inst.engine
                            ev.sync_info = mybir.SyncInfo(
                                on_wait=chunk, on_update=[]
                            )
                            out.append(ev)
                            k += 1
                        inst.sync_info = mybir.SyncInfo(
                            on_wait=keep, on_update=upds
                        )
                        changed = True
                out.append(inst)
            if changed:
                il[:] = out
    return nc


def _rep2(ap_2d, n):
    """View a [128, F] AP as [128, n, F] with a step-0 middle dim."""
    return bass.AP(
        tensor=ap_2d.tensor,
        offset=ap_2d.offset,
        ap=[ap_2d.ap[0], [0, n], ap_2d.ap[1]],
    )


def _wbc_src(wt_ap, b_shard, b0, nb):
    """DRAM AP [128(bcast), E, nb]: every partition reads all E weight
    rows for the tile's token slice."""
    return bass.AP(
        tensor=wt_ap.tensor,
        offset=b0,
        ap=[[0, 128], [b_shard, E], [1, nb]],
    )


def build_nc(b_shard=B_SHARD, nb=NB, legalize=True):
    assert b_shard % nb == 0
    n_tiles = b_shard // nb
    nc = bass.Bass(trn_type="TRN2")

    xt = nc.dram_tensor("xt", [D_IN, b_shard], BF16, kind="ExternalInput").ap()
    wt = nc.dram_tensor("wt", [E, b_shard], BF16, kind="ExternalInput").ap()
    # W1 laid out [i, (e, m), h']: chunk (e, m) is lhsT for z_e rows m*128..
    w1l = nc.dram_tensor("w1l", [D_IN, E * NCHUNK, 128], BF16, kind="ExternalInput").ap()
    # b1 laid out [p, (e, m)] = b1[e, m*128 + p]; b1n = negated copy
    b1l = nc.dram_tensor("b1l", [128, E * NCHUNK], F32, kind="ExternalInput").ap()
    b1n = nc.dram_tensor("b1n", [128, E * NCHUNK], F32, kind="ExternalInput").ap()
    # W2 laid out [h', (e, k), o]: chunk (e, k) is lhsT contracting h rows k*128..
    w2l = nc.dram_tensor("w2l", [128, E * NCHUNK, D_OUT], BF16, kind="ExternalInput").ap()
    # b2p = b2 + W2^T b1 for fused experts (host-folded correction)
    b2p = nc.dram_tensor("b2p", [E, D_OUT], BF16, kind="ExternalInput").ap()
    outT = nc.dram_tensor("outT", [D_OUT, b_shard], F32, kind="ExternalOutput").ap()

    nsub = nb // 512  # matmul moving-operand splits per tile
    with tile.TileContext(nc) as tc:
        with (
            tc.tile_pool(name="consts", bufs=1) as consts,
            tc.tile_pool(name="xt_p", bufs=3) as xt_p,
            tc.tile_pool(name="wt_p", bufs=3) as wt_p,
            tc.tile_pool(name="wbc_p", bufs=3) as wbc_p,
            tc.tile_pool(name="h_p", bufs=6) as h_p,
            tc.tile_pool(name="hs_p", bufs=6) as hs_p,
            tc.tile_pool(name="ot_p", bufs=3) as ot_p,
            # PSUM budget: 8 banks of 2KB. z gets 3 full tiles (6 banks);
            # the output accumulator is 2 independent half-tiles (1 bank
            # each) so consecutive tiles' accumulations can overlap.
            tc.tile_pool(name="z_ps", bufs=3, space="PSUM") as z_ps,
            tc.tile_pool(name="o_ps", bufs=2, space="PSUM") as o_ps,
        ):
            # Const DMA queue layout, tuned for time-to-first-work:
            #   scalar: b1 (tiny) -> b1 launder copy -> relus start early
            #   gpsimd: w1 in pieces (first piece smallest), then w2/b1n,
            #           then the per-tile wbc broadcasts
            #   sync:   b2' (tiny), then the per-tile xt/wt/out stream
            b1_dma = consts.tile([128, E * NCHUNK], F32, tag="b1_dma")
            nc.scalar.dma_start(b1_dma, b1l)
            b1_sb = consts.tile([128, E * NCHUNK], F32, tag="b1_act")
            nc.scalar.copy(b1_sb, b1_dma)
            w1_sb = consts.tile([D_IN, E * NCHUNK, 128], BF16)
            for lo, hi in ((0, 2), (2, 4), (4, 8), (8, 12), (12, 16)):
                nc.gpsimd.dma_start(w1_sb[:, lo:hi, :], w1l[:, lo:hi, :])
            w2_sb = consts.tile([128, E * NCHUNK, D_OUT], BF16)
            nc.gpsimd.dma_start(w2_sb, w2l)
            b1n_dma = consts.tile([128, E * NCHUNK], F32, tag="b1n_dma")
            nc.gpsimd.dma_start(b1n_dma, b1n)
            b1n_sb = consts.tile([128, E * NCHUNK], F32, tag="b1n_dve")
            nc.vector.tensor_copy(b1n_sb, b1n_dma)
            # b2' rides the scalar queue so the sync queue's first transfer
            # is tile-0's xt (which gates the very first matmul)
            b2_sb = consts.tile([E, D_OUT], BF16)
            nc.scalar.dma_start(b2_sb, b2p)

            # Pre-warm the PE while the head is DMA-bound: the HAM clock
            # gate starts at 1.2 GHz and needs ~3.4us of sustained matmul
            # activity to release to 2.4 GHz. Burn that window on dummy
            # matmuls over a memset tile so tile 0's real matmuls run warm.
            if N_WARM_MM:
                warm = consts.tile([128, 128], BF16, tag="warm")
                nc.vector.memset(warm, 0)
                zw = z_ps.tile([128, nb], F32, tag="z")
                for _ in range(N_WARM_MM):
                    nc.tensor.matmul(
                        zw[:, :128], lhsT=warm, rhs=warm,
                        start=True, stop=True,
                    )

            for t in range(n_tiles):
                b0 = t * nb
                xt_sb = xt_p.tile([D_IN, nb], BF16)
                if t == 0:
                    # tile 0 gates the very first matmul: land the first
                    # 512-token half sooner with a split transfer
                    for j in range(nsub):
                        sl = slice(j * 512, (j + 1) * 512)
                        nc.sync.dma_start(xt_sb[:, sl], xt[:, b0 + j * 512 : b0 + (j + 1) * 512])
                else:
                    nc.sync.dma_start(xt_sb, xt[:, b0 : b0 + nb])
                wt_sb = wt_p.tile([E, nb], BF16)
                nc.sync.dma_start(wt_sb, wt[:, b0 : b0 + nb])
                # broadcast weight rows to 128 partitions (per-expert DMAs,
                # spread in time so SBUF write bursts don't starve engines)
                wbc = wbc_p.tile([128, E, nb], BF16)
                for e in range(E):
                    nc.gpsimd.dma_start(
                        wbc[:, e, :],
                        wt[e : e + 1, b0 : b0 + nb].partition_broadcast(128),
                    )

                # out^T accumulates per 512-token half in its own PSUM bank;
                # the b2' seed matmuls are emitted after expert 0's z work so
                # the PE has useful work while the previous tile's bank drains
                pos = []
                for j in range(nsub):
                    po = o_ps.tile([D_OUT, 512], F32, tag="po")
                    pos.append(po)

                def flush(pe, phs, ph, pact_ms):
                    # multiply + L2 for an expert whose relus were emitted
                    # one expert ago: the DVE's strict FIFO never blocks a
                    # z-drain relu behind a mult still waiting on ACT / the
                    # broadcast DMA, and the h/wbc deps get a full expert
                    # period of slack.
                    for m in pact_ms:
                        eng = nc.gpsimd if pe in GP_MULT_EXPERTS else nc.vector
                        eng.tensor_mul(
                            phs[:, m, :], ph[:, m, :], wbc[:, pe, :]
                        )
                    for k in range(NCHUNK):
                        c = NCHUNK * pe + k
                        for j in range(nsub):
                            sl = slice(j * 512, (j + 1) * 512)
                            nc.tensor.matmul(
                                pos[j], lhsT=w2_sb[:, c, :], rhs=phs[:, k, sl],
                                start=False,
                                stop=(pe == E - 1 and k == NCHUNK - 1),
                            )

                pend = None
                for e in range(E):
                    hs = hs_p.tile([128, NCHUNK, nb], BF16)
                    h = h_p.tile([128, NCHUNK, nb], BF16)
                    act_ms = []
                    for m in range(NCHUNK):
                        c = NCHUNK * e + m
                        z = z_ps.tile([128, nb], F32, tag="z")
                        for j in range(nsub):
                            sl = slice(j * 512, (j + 1) * 512)
                            nc.tensor.matmul(
                                z[:, sl], lhsT=w1_sb[:, c, :], rhs=xt_sb[:, sl],
                                start=True, stop=True,
                            )
                        if c in FUSED_CHUNKS:
                            # g = max(z, -b1) * w  in one DVE pass; the
                            # missing w*b1 term is folded into b2' on host
                            nc.vector.scalar_tensor_tensor(
                                hs[:, m, :], z, b1n_sb[:, c : c + 1],
                                wbc[:, e, :], MAX, MULT,
                            )
                        elif c in DVE_RELU_CHUNKS:
                            act_ms.append(m)
                            # DVE relu: (z + b1) max 0, cast to bf16
                            nc.vector.tensor_scalar(
                                h[:, m, :], z,
                                b1_sb[:, c : c + 1], 0.0,
                                mybir.AluOpType.add, mybir.AluOpType.max,
                            )
                        else:
                            act_ms.append(m)
                            nc.scalar.activation(
                                h[:, m, :], z, RELU,
                                bias=b1_sb[:, c : c + 1], scale=1.0,
                            )
                    if e == 0:
                        # out^T := b2'^T @ w^T   (K = 8), opens the group
                        for j in range(nsub):
                            sl = slice(j * 512, (j + 1) * 512)
                            nc.tensor.matmul(
                                pos[j], lhsT=b2_sb, rhs=wt_sb[:, sl],
                                start=True, stop=False,
                            )
                    if pend is not None:
                        flush(*pend)
                    pend = (e, hs, h, act_ms)
                flush(*pend)

                ot = ot_p.tile([D_OUT, nb], F32)
                for j in range(nsub):
                    sl = slice(j * 512, (j + 1) * 512)
                    # drain the two PSUM halves on different engines so the
                    # tile-boundary copy never queues behind ACT's relus
                    if j == 0:
                        nc.scalar.copy(ot[:, sl], pos[j])
                    else:
                        nc.vector.tensor_copy(ot[:, sl], pos[j])
                    nc.sync.dma_start(outT[:, b0 + j * 512 : b0 + (j + 1) * 512], ot[:, sl])
    dedup_ldw(nc)
    return legalize_waits(nc) if legalize else nc


def prep_consts(W1, b1, W2, b2):
    bf = ml_dtypes.bfloat16
    # w1l[i, (e, m), h'] = W1[e, i, m*128 + h']
    w1l = np.ascontiguousarray(
        W1.transpose(1, 0, 2).reshape(D_IN, E, NCHUNK, 128).reshape(D_IN, E * NCHUNK, 128)
    ).astype(bf)
    # b1l[p, (e, m)] = b1[e, m*128 + p]
    b1l = np.ascontiguousarray(
        b1.reshape(E, NCHUNK, 128).transpose(2, 0, 1).reshape(128, E * NCHUNK)
    ).astype(np.float32)
    # w2l[h', (e, k), o] = W2[e, k*128 + h', o]
    w2l = np.ascontiguousarray(
        W2.reshape(E, NCHUNK, 128, D_OUT).transpose(2, 0, 1, 3).reshape(128, E * NCHUNK, D_OUT)
    ).astype(bf)
    # b2' = b2 + W2^T b1 over the fused chunks' h-ranges: the fused DVE op
    # computes w*max(z,-b1) = w*relu(z+b1) - w*b1; the missing w*b1 term
    # passes through L2 linearly as (W2^T b1) (x) w and lands in the combine.
    b2p = b2.astype(np.float64).copy()
    for c in FUSED_CHUNKS:
        e, m = divmod(c, NCHUNK)
        hr = slice(m * 128, (m + 1) * 128)
        b2p[e] += np.einsum(
            "ho,h->o", W2[e, hr].astype(np.float64), b1[e, hr].astype(np.float64)
        )
    return {
        "w1l": w1l,
        "b1l": b1l,
        "b1n": -b1l,
        "w2l": w2l,
        "b2p": b2p.astype(np.float32).astype(bf),
    }


def prep_core(x_c, w_c, consts, b_shard):
    bf = ml_dtypes.bfloat16
    xt = np.ascontiguousarray(x_c.T).astype(bf)
    wt = np.ascontiguousarray(w_c.T).astype(bf)
    return {"xt": xt, "wt": wt, **consts}


def _ntff_hook():
    """NTFF profiling hook via the axon PJRT .so (the antenv.axon_hooks
    glue module is absent in this image, so wire it up directly)."""
    from trn_agent_boot.trn_boot import _ntff_profile_via_ctypes

    return _ntff_profile_via_ctypes("/opt/axon/libaxon_pjrt.so")


def run_traced(nc, in_maps, n_cores, out_dir):
    import concourse.bass2jax as bass2jax

    hook = _ntff_hook()
    with hook(out_dir, list(range(n_cores))):
        results = bass2jax.run_bass_via_pjrt(nc, in_maps, n_cores=n_cores)
    return results


def run(inputs, trace=False, b_shard=B_SHARD, nb=NB):
    x = np.asarray(inputs["x"], dtype=np.float32)
    w = np.asarray(inputs["weights"], dtype=np.float32)
    consts = prep_consts(
        np.asarray(inputs["W1"], dtype=np.float32),
        np.asarray(inputs["b1"], dtype=np.float32),
        np.asarray(inputs["W2"], dtype=np.float32),
        np.asarray(inputs["b2"], dtype=np.float32),
    )
    n_cores = x.shape[0] // b_shard
    key = (b_shard, nb)
    if key not in _nc_cache:
        _nc_cache[key] = build_nc(b_shard, nb)
    nc = _nc_cache[key]
    in_maps = [
        prep_core(
            x[c * b_shard : (c + 1) * b_shard],
            w[c * b_shard : (c + 1) * b_shard],
            consts,
            b_shard,
        )
        for c in range(n_cores)
    ]
    if trace:
        import tempfile

        out_dir = tempfile.mkdtemp(prefix="moe_ntff_")
        results = run_traced(nc, in_maps, n_cores, out_dir)

        class _Res:
            pass

        res = _Res()
        res.results = results
        res.exec_time_ns = None
        res.trace_dir = out_dir
    else:
        res = run_bass_kernel_spmd(
            nc, in_maps, core_ids=list(range(n_cores)), trace=False
        )
        res.trace_dir = None
    out = np.concatenate([np.ascontiguousarray(r["outT"].T) for r in res.results], axis=0)
    return out.astype(np.float32), res


def kernel(**inputs) -> np.ndarray:
    out, _ = run(inputs)
    return out


# revision 46
# speedup vs baseline: 1.0625x; 1.0625x over previous
# MoE layer (all-experts dense MLP + weighted combine) on 8 TRN2 NeuronCores.
#
# Reference, for every token b (B=65536 total):
#   h_e   = relu(x @ W1[e] + b1[e])          e = 0..7
#   y_e   = h_e @ W2[e] + b2[e]
#   out_b = sum_e weights[b, e] * y_e
#
# Strategy (data-parallel over B, expert params replicated):
#   - Shard B across the 8 cores (8192 tokens each).
#   - Hidden dim stays on partitions ("hdim-major"):
#       L1:  z_e^T[h, b]  = W1_e^T @ x^T          (W1 chunks stationary)
#       h_e^T             = relu(z_e^T + b1_e)    (split ACT/DVE)
#       hs_e^T            = h_e^T * w_bcast_e     (DVE, per chunk)
#       out^T[o, b]       = sum_{e,k} W2_chunk^T @ hs_chunk + b2^T @ w^T
#     accumulated in two per-half PSUM banks - the expert combine is free
#     and consecutive tiles' accumulations overlap.
#   - w broadcast to 128 partitions via per-expert DMAs with a step-0
#     partition AP reading the host-transposed weights row from DRAM.
#     The relu ops deliberately depend only on z + b1 (no DMA), so the
#     z-PSUM drain never waits on the broadcast stream.
#   - Head/tail tuned: consts ride the scalar/gpsimd queues (w1 in pieces)
#     so tile-0's xt is the sync queue's first transfer; dummy matmuls in
#     the DMA-bound head release the HAM clock gate before real work; the
#     b2 seed matmuls are emitted after expert 0's z work.
#   - Host-side prep: x / weights transposed + cast to bf16, expert params
#     in matmul-ready layout; output produced transposed and un-transposed
#     on the host. All device work is the unavoidable compute.
import numpy as np
import ml_dtypes

import concourse.bass as bass
import concourse.mybir as mybir
import concourse.tile as tile
import concourse.bass_utils as _bu
from concourse.bass_utils import run_bass_kernel_spmd


E, D_IN, D_HID, D_OUT, B = 8, 128, 256, 128, 65536
N_CORES = 8
B_SHARD = B // N_CORES  # 8192
NB = 1024               # tokens per tile
NCHUNK = D_HID // 128   # 2 hidden-dim chunks per expert

BF16 = mybir.dt.bfloat16
F32 = mybir.dt.float32
RELU = mybir.ActivationFunctionType.Relu
MAX = mybir.AluOpType.max
MULT = mybir.AluOpType.mult

# Engine split knobs (tuned against the profile so every engine sits
# below the PE's ~14.3us/tile floor):
#  - FUSED_CHUNKS (c = 2*e + m): relu+weighting in one DVE
#    scalar_tensor_tensor, with the w*b1 term host-folded into b2'.
#    NOTE: measured counterproductive — the fused op puts the wbc DMA on
#    the critical z-drain path and the pipeline de-syncs (+33us). Keep ().
#  - DVE_RELU_CHUNKS: relu on DVE via tensor_scalar (z + b1 deps only).
#  - remaining chunks: ACT relu; per-expert weight multiply on DVE.
#  - GP_MULT_EXPERTS: multiplies offloaded to GPSIMD: measured
#    counterproductive (blocks wbc triggers in the gpsimd FIFO). Keep ().
FUSED_CHUNKS = ()
DVE_RELU_CHUNKS = (1, 5, 9, 13)
GP_MULT_EXPERTS = ()
N_WARM_MM = 28  # dummy matmuls in the DMA-bound head to spin up HAM

_nc_cache = {}


def dedup_ldw(nc):
    """Drop redundant PE weight loads.

    Tile emits an InstLdweights before every InstMatmult; consecutive
    matmuls over the two 512-token halves of a tile reuse the same
    stationary weights, so the second load is a hardware no-op (weights
    persist in the PE array until the next load). Deleting it saves PE
    queue time; its semaphore waits/updates are carried onto the next PE
    instruction (legalize_waits splits any overflow afterwards).
    """
    for f in nc.m.functions:
        for b in f.blocks:
            il = b.instructions
            out = []
            last_key = None
            carry_w, carry_u = [], []
            for inst in il:
                if inst.engine != mybir.EngineType.PE:
                    out.append(inst)
                    continue
                if isinstance(inst, mybir.InstLdweights):
                    key = str(inst.ins[0])
                    if key == last_key:
                        si = inst.sync_info
                        if si is not None:
                            carry_w.extend(list(si.on_wait))
                            carry_u.extend(list(si.on_update))
                        continue
                    last_key = key
                elif not isinstance(
                    inst, (mybir.InstMatmult, mybir.InstEventSemaphore)
                ):
                    last_key = None
                if carry_w or carry_u:
                    si = inst.sync_info
                    w = (list(si.on_wait) if si else []) + carry_w
                    u = (list(si.on_update) if si else []) + carry_u
                    inst.sync_info = mybir.SyncInfo(on_wait=w, on_update=u)
                    carry_w, carry_u = [], []
                out.append(inst)
            il[:] = out
    return nc


def legalize_waits(nc):
    """Split multi-wait instructions into standalone EventSemaphore waits.

    The walrus build in this container enforces the hardware sync-slot
    budget strictly: a normal instruction holds at most 1 sem wait (+1
    update); an EventSemaphore instruction holds 2. Tile's scheduler
    attaches up to 3 waits per instruction (and ~11 on the kernel-tail
    drain), which codegen rejects with "Too many sync wait commands".
    Hoisting the excess waits into standalone EventSemaphore instructions
    immediately before the op (same engine queue, so they gate execution
    identically) makes the program legal without changing semantics.
    """
    for f in nc.m.functions:
        for b in f.blocks:
            il = b.instructions
            out = []
            changed = False
            for inst in il:
                si = inst.sync_info
                if si is not None:
                    waits = list(si.on_wait)
                    upds = list(si.on_update)
                    assert len(upds) <= 1, f"{inst.name}: {len(upds)} updates"
                    cap = 2 if isinstance(inst, mybir.InstEventSemaphore) else 1
                    if len(waits) > cap:
                        extra, keep = waits[:-cap], waits[-cap:]
                        k = 0
                        while extra:
                            chunk, extra = extra[:2], extra[2:]
                            ev = mybir.InstEventSemaphore(
                                name=f"{inst.name}-lw{k}", ins=[], outs=[]
                            )
                            ev.engine = inst.engine
                            ev.sync_info = mybir.SyncInfo(
                                on_wait=chunk, on_update=[]
                            )
                            out.append(ev)
                            k += 1
                        inst.sync_info = mybir.SyncInfo(
                            on_wait=keep, on_update=upds
                        )
                        changed = True
                out.append(inst)
            if changed:
                il[:] = out
    return nc


def _rep2(ap_2d, n):
    """View a [128, F] AP as [128, n, F] with a step-0 middle dim."""
    return bass.AP(
        tensor=ap_2d.tensor,
        offset=ap_2d.offset,
        ap=[ap_2d.ap[0], [0, n], ap_2d.ap[1]],
    )


def _wbc_src(wt_ap, b_shard, b0, nb):
    """DRAM AP [128(bcast), E, nb]: every partition reads all E weight
    rows for the tile's token slice."""
    return bass.AP(
        tensor=wt_ap.tensor,
        offset=b0,
        ap=[[0, 128], [b_shard, E], [1, nb]],
    )


def build_nc(b_shard=B_SHARD, nb=NB, legalize=True):
    assert b_shard % nb == 0
    n_tiles = b_shard // nb
    nc = bass.Bass(trn_type="TRN2")

    xt = nc.dram_tensor("xt", [D_IN, b_shard], BF16, kind="ExternalInput").ap()
    wt = nc.dram_tensor("wt", [E, b_shard], BF16, kind="ExternalInput").ap()
    # W1 laid out [i, (e, m), h']: chunk (e, m) is lhsT for z_e rows m*128..
    w1l = nc.dram_tensor("w1l", [D_IN, E * NCHUNK, 128], BF16, kind="ExternalInput").ap()
    # b1 laid out [p, (e, m)] = b1[e, m*128 + p]; b1n = negated copy
    b1l = nc.dram_tensor("b1l", [128, E * NCHUNK], F32, kind="ExternalInput").ap()
    b1n = nc.dram_tensor("b1n", [128, E * NCHUNK], F32, kind="ExternalInput").ap()
    # W2 laid out [h', (e, k), o]: chunk (e, k) is lhsT contracting h rows k*128..
    w2l = nc.dram_tensor("w2l", [128, E * NCHUNK, D_OUT], BF16, kind="ExternalInput").ap()
    # b2p = b2 + W2^T b1 for fused experts (host-folded correction)
    b2p = nc.dram_tensor("b2p", [E, D_OUT], BF16, kind="ExternalInput").ap()
    outT = nc.dram_tensor("outT", [D_OUT, b_shard], F32, kind="ExternalOutput").ap()

    nsub = nb // 512  # matmul moving-operand splits per tile
    with tile.TileContext(nc) as tc:
        with (
            tc.tile_pool(name="consts", bufs=1) as consts,
            tc.tile_pool(name="xt_p", bufs=3) as xt_p,
            tc.tile_pool(name="wt_p", bufs=3) as wt_p,
            tc.tile_pool(name="wbc_p", bufs=3) as wbc_p,
            tc.tile_pool(name="h_p", bufs=8) as h_p,
            tc.tile_pool(name="hs_p", bufs=8) as hs_p,
            tc.tile_pool(name="ot_p", bufs=3) as ot_p,
            # PSUM budget: 8 banks of 2KB. z gets 3 full tiles (6 banks);
            # the output accumulator is 2 independent half-tiles (1 bank
            # each) so consecutive tiles' accumulations can overlap.
            tc.tile_pool(name="z_ps", bufs=3, space="PSUM") as z_ps,
            tc.tile_pool(name="o_ps", bufs=2, space="PSUM") as o_ps,
        ):
            # Const DMA queue layout, tuned for time-to-first-work:
            #   scalar: b1 (tiny) -> b1 launder copy -> relus start early
            #   gpsimd: w1 in pieces (first piece smallest), then w2/b1n,
            #           then the per-tile wbc broadcasts
            #   sync:   b2' (tiny), then the per-tile xt/wt/out stream
            b1_dma = consts.tile([128, E * NCHUNK], F32, tag="b1_dma")
            nc.scalar.dma_start(b1_dma, b1l)
            b1_sb = consts.tile([128, E * NCHUNK], F32, tag="b1_act")
            nc.scalar.copy(b1_sb, b1_dma)
            b1v_sb = consts.tile([128, E * NCHUNK], F32, tag="b1_dve")
            nc.vector.tensor_copy(b1v_sb, b1_dma)
            w1_sb = consts.tile([D_IN, E * NCHUNK, 128], BF16)
            for lo, hi in ((0, 2), (2, 4), (4, 8), (8, 12), (12, 16)):
                nc.gpsimd.dma_start(w1_sb[:, lo:hi, :], w1l[:, lo:hi, :])
            w2_sb = consts.tile([128, E * NCHUNK, D_OUT], BF16)
            nc.gpsimd.dma_start(w2_sb, w2l)
            b1n_dma = consts.tile([128, E * NCHUNK], F32, tag="b1n_dma")
            nc.gpsimd.dma_start(b1n_dma, b1n)
            b1n_sb = consts.tile([128, E * NCHUNK], F32, tag="b1n_dve")
            nc.vector.tensor_copy(b1n_sb, b1n_dma)
            # b2' rides the scalar queue so the sync queue's first transfer
            # is tile-0's xt (which gates the very first matmul)
            b2_sb = consts.tile([E, D_OUT], BF16)
            nc.scalar.dma_start(b2_sb, b2p)

            # Pre-warm the PE while the head is DMA-bound: the HAM clock
            # gate starts at 1.2 GHz and needs ~3.4us of sustained matmul
            # activity to release to 2.4 GHz. Burn that window on dummy
            # matmuls over a memset tile so tile 0's real matmuls run warm.
            if N_WARM_MM:
                warm = consts.tile([128, 128], BF16, tag="warm")
                nc.vector.memset(warm, 0)
                zw = z_ps.tile([128, nb], F32, tag="z")
                for _ in range(N_WARM_MM):
                    nc.tensor.matmul(
                        zw[:, :128], lhsT=warm, rhs=warm,
                        start=True, stop=True,
                    )

            for t in range(n_tiles):
                b0 = t * nb
                xt_sb = xt_p.tile([D_IN, nb], BF16)
                if t == 0:
                    # tile 0 gates the very first matmul: land the first
                    # 512-token half sooner with a split transfer
                    for j in range(nsub):
                        sl = slice(j * 512, (j + 1) * 512)
                        nc.sync.dma_start(xt_sb[:, sl], xt[:, b0 + j * 512 : b0 + (j + 1) * 512])
                else:
                    nc.sync.dma_start(xt_sb, xt[:, b0 : b0 + nb])
                wt_sb = wt_p.tile([E, nb], BF16)
                nc.sync.dma_start(wt_sb, wt[:, b0 : b0 + nb])
                # broadcast weight rows to 128 partitions (per-expert DMAs,
                # spread in time so SBUF write bursts don't starve engines)
                wbc = wbc_p.tile([128, E, nb], BF16)
                for e in range(E):
                    nc.gpsimd.dma_start(
                        wbc[:, e, :],
                        wt[e : e + 1, b0 : b0 + nb].partition_broadcast(128),
                    )

                # out^T accumulates per 512-token half in its own PSUM bank;
                # the b2' seed matmuls are emitted after expert 0's z work so
                # the PE has useful work while the previous tile's bank drains
                pos = []
                for j in range(nsub):
                    po = o_ps.tile([D_OUT, 512], F32, tag="po")
                    pos.append(po)

                def flush(pe, phs, ph, pact_ms):
                    # multiply + L2 for an expert whose relus were emitted
                    # one expert ago: the DVE's strict FIFO never blocks a
                    # z-drain relu behind a mult still waiting on ACT / the
                    # broadcast DMA, and the h/wbc deps get a full expert
                    # period of slack.
                    for m in pact_ms:
                        eng = nc.gpsimd if pe in GP_MULT_EXPERTS else nc.vector
                        eng.tensor_mul(
                            phs[:, m, :], ph[:, m, :], wbc[:, pe, :]
                        )
                    for k in range(NCHUNK):
                        c = NCHUNK * pe + k
                        for j in range(nsub):
                            sl = slice(j * 512, (j + 1) * 512)
                            nc.tensor.matmul(
                                pos[j], lhsT=w2_sb[:, c, :], rhs=phs[:, k, sl],
                                start=False,
                                stop=(pe == E - 1 and k == NCHUNK - 1),
                            )

                pend = None
                for e in range(E):
                    hs = hs_p.tile([128, NCHUNK, nb], BF16)
                    h = h_p.tile([128, NCHUNK, nb], BF16)
                    act_ms = []
                    for m in range(NCHUNK):
                        c = NCHUNK * e + m
                        z = z_ps.tile([128, nb], F32, tag="z")
                        for j in range(nsub):
                            sl = slice(j * 512, (j + 1) * 512)
                            nc.tensor.matmul(
                                z[:, sl], lhsT=w1_sb[:, c, :], rhs=xt_sb[:, sl],
                                start=True, stop=True,
                            )
                        if c in FUSED_CHUNKS:
                            # g = max(z, -b1) * w  in one DVE pass; the
                            # missing w*b1 term is folded into b2' on host
                            nc.vector.scalar_tensor_tensor(
                                hs[:, m, :], z, b1n_sb[:, c : c + 1],
                                wbc[:, e, :], MAX, MULT,
                            )
                        elif c in DVE_RELU_CHUNKS:
                            act_ms.append(m)
                            # DVE relu: (z + b1) max 0, cast to bf16
                            nc.vector.tensor_scalar(
                                h[:, m, :], z,
                                b1v_sb[:, c : c + 1], 0.0,
                                mybir.AluOpType.add, mybir.AluOpType.max,
                            )
                        else:
                            act_ms.append(m)
                            nc.scalar.activation(
                                h[:, m, :], z, RELU,
                                bias=b1_sb[:, c : c + 1], scale=1.0,
                            )
                    if e == 0:
                        # out^T := b2'^T @ w^T   (K = 8), opens the group
                        for j in range(nsub):
                            sl = slice(j * 512, (j + 1) * 512)
                            nc.tensor.matmul(
                                pos[j], lhsT=b2_sb, rhs=wt_sb[:, sl],
                                start=True, stop=False,
                            )
                    if pend is not None:
                        flush(*pend)
                    pend = (e, hs, h, act_ms)
                flush(*pend)

                ot = ot_p.tile([D_OUT, nb], F32)
                for j in range(nsub):
                    sl = slice(j * 512, (j + 1) * 512)
                    nc.scalar.copy(ot[:, sl], pos[j])
                    nc.sync.dma_start(outT[:, b0 + j * 512 : b0 + (j + 1) * 512], ot[:, sl])
    dedup_ldw(nc)
    return legalize_waits(nc) if legalize else nc


def prep_consts(W1, b1, W2, b2):
    bf = ml_dtypes.bfloat16
    # w1l[i, (e, m), h'] = W1[e, i, m*128 + h']
    w1l = np.ascontiguousarray(
        W1.transpose(1, 0, 2).reshape(D_IN, E, NCHUNK, 128).reshape(D_IN, E * NCHUNK, 128)
    ).astype(bf)
    # b1l[p, (e, m)] = b1[e, m*128 + p]
    b1l = np.ascontiguousarray(
        b1.reshape(E, NCHUNK, 128).transpose(2, 0, 1).reshape(128, E * NCHUNK)
    ).astype(np.float32)
    # w2l[h', (e, k), o] = W2[e, k*128 + h', o]
    w2l = np.ascontiguousarray(
        W2.reshape(E, NCHUNK, 128, D_OUT).transpose(2, 0, 1, 3).reshape(128, E * NCHUNK, D_OUT)
    ).astype(bf)
    # b2' = b2 + W2^T b1 over the fused chunks' h-ranges: the fused DVE op
    # computes w*max(z,-b1) = w*relu(z+b1) - w*b1; the missing w*b1 term
    # passes through L2 linearly as (W2^T b1) (x) w and lands in the combine.
    b2p = b2.astype(np.float64).copy()
    for c in FUSED_CHUNKS:
        e, m = divmod(c, NCHUNK)
        hr = slice(m * 128, (m + 1) * 128)
        b2p[e] += np.einsum(
            "ho,h->o", W2[e, hr].astype(np.float64), b1[e, hr].astype(np.float64)
        )
    return {
        "w1l": w1l,
        "b1l": b1l,
        "b1n": -b1l,
        "w2l": w2l,
        "b2p": b2p.astype(np.float32).astype(bf),
    }


def prep_core(x_c, w_c, consts, b_shard):
    bf = ml_dtypes.bfloat16
    xt = np.ascontiguousarray(x_c.T).astype(bf)
    wt = np.ascontiguousarray(w_c.T).astype(bf)
    return {"xt": xt, "wt": wt, **consts}


def _ntff_hook():
    """NTFF profiling hook via the axon PJRT .so (the antenv.axon_hooks
    glue module is absent in this image, so wire it up directly)."""
    from trn_agent_boot.trn_boot import _ntff_profile_via_ctypes

    return _ntff_profile_via_ctypes("/opt/axon/libaxon_pjrt.so")


def run_traced(nc, in_maps, n_cores, out_dir):
    import concourse.bass2jax as bass2jax

    hook = _ntff_hook()
    with hook(out_dir, list(range(n_cores))):
        results = bass2jax.run_bass_via_pjrt(nc, in_maps, n_cores=n_cores)
    return results


def run(inputs, trace=False, b_shard=B_SHARD, nb=NB):
    x = np.asarray(inputs["x"], dtype=np.float32)
    w = np.asarray(inputs["weights"], dtype=np.float32)
    consts = prep_consts(
        np.asarray(inputs["W1"], dtype=np.float32),
        np.asarray(inputs["b1"], dtype=np.float32),
        np.asarray(inputs["W2"], dtype=np.float32),
        np.asarray(inputs["b2"], dtype=np.float32),
    )
    n_cores = x.shape[0] // b_shard
    key = (b_shard, nb)
    if key not in _nc_cache:
        _nc_cache[key] = build_nc(b_shard, nb)
    nc = _nc_cache[key]
    in_maps = [
        prep_core(
            x[c * b_shard : (c + 1) * b_shard],
            w[c * b_shard : (c + 1) * b_shard],
            consts,
            b_shard,
        )
        for c in range(n_cores)
    ]
    if trace:
        import tempfile

        out_dir = tempfile.mkdtemp(prefix="moe_ntff_")
        results = run_traced(nc, in_maps, n_cores, out_dir)

        class _Res:
            pass

        res = _Res()
        res.results = results
        res.exec_time_ns = None
        res.trace_dir = out_dir
    else:
        res = run_bass_kernel_spmd(
            nc, in_maps, core_ids=list(range(n_cores)), trace=False
        )
        res.trace_dir = None
    out = np.concatenate([np.ascontiguousarray(r["outT"].T) for r in res.results], axis=0)
    return out.astype(np.float32), res


def kernel(**inputs) -> np.ndarray:
    out, _ = run(inputs)
    return out
